# revision 19
# baseline (speedup 1.0000x reference)
# Trainium2 Bass kernel for:
#   q = x @ Wq.T + bq ; k = x @ Wk.T + bk ; v = x @ Wv.T + bv
#   g = sigmoid(x @ Wg.T + bg)
#   out = q * cumsum(k*v, axis=seq) * g
#
# Sharding: tensor-parallel split of the 2048 output features across the 8
# cores (256 features each). All ops are per-feature except the d-contraction
# (each core uses the full x) and the cumsum along seq (handled fully on-core
# per (batch, feature)) -> zero cross-core communication.
#
# On-core layout is [e, t] (features on partitions, tokens on the free dim):
#   - linears:  psum[e,t] += W_chunk.T @ x_chunk   (bf16 matmuls, fp32 accum)
#   - bias:     ACT activation Identity with per-partition bias (bf16 out)
#   - sigmoid:  ACT activation with per-partition bias (bf16 out)
#   - cumsum:   DVE tensor_tensor_scan along the free dim (fp32 state/out),
#               chained across token (sub)tiles via initial=prev[:, -1:]
#   - qg mul on the Pool engine, kv/out muls on DVE.
# The host pre-packs x into unit tiles [B, NU, 128p, KC, TT] (loaded in
# 4-chunk quarters) and W into [128p, KC, E] so every DMA row is one long
# contiguous packet (4KB for x quarters, 2KB for W quarters), and transposes
# the [B, E, S] per-core bf16 outputs back to fp32 at the end. The final unit
# is processed in 128-token sub-tiles to shorten the post-matmul drain chain.

from contextlib import ExitStack

import numpy as np
import ml_dtypes

import concourse.bass as bass  # noqa: F401  (bass types referenced via tile/bacc)
import concourse.tile as tile
from concourse import bacc, mybir
from concourse.bass_utils import run_bass_kernel_spmd

N_CORES = 8
B, S, D = 4, 4096, 2048
E = D // N_CORES  # 256 output features per core
TT = 512          # token tile (free dim of psum)
KC = D // 128     # contraction chunks
XJ = 4            # chunks per x quarter-tile
NXJ = KC // XJ    # x quarter-tiles per unit
NU = S // TT      # token tiles per batch
MH = E // 128     # feature halves (psum groups per linear)
MM_DT = mybir.dt.bfloat16
MM_NP = ml_dtypes.bfloat16


def build_nc(b=B, s=S, d=D, e=E, tt=TT, mm_dt=MM_DT, n_cores=N_CORES):
    kc = KC
    nu = NU
    mh = MH
    f32 = mybir.dt.float32
    names = "qkvg"

    nc = bacc.Bacc(
        "TRN2", target_bir_lowering=False, debug=False, num_devices=n_cores
    )
    # x packed on host: X5[b, n, p, j, c, t] = x[b, n*tt+t, (j*4+c)*128+p]
    # (partition-major so a whole unit [128, kc, tt] is one contiguous-row DMA)
    X5 = nc.dram_tensor(
        "X5", [b, nu, 128, NXJ, XJ, tt], mm_dt, kind="ExternalInput"
    ).ap()
    # W packed on host: W5[p, c, e] = W[core_sl][e, c*128+p]
    W5 = {
        x_: nc.dram_tensor(f"W{x_}5", [128, kc, e], mm_dt, kind="ExternalInput").ap()
        for x_ in names
    }
    bias = {
        x_: nc.dram_tensor(f"b{x_}", [e], f32, kind="ExternalInput").ap()
        for x_ in names
    }
    outT = nc.dram_tensor("outT", [b, e, s], mybir.dt.bfloat16, kind="ExternalOutput").ap()

    add = mybir.AluOpType.add
    bypass = mybir.AluOpType.bypass
    mult = mybir.AluOpType.mult
    sigmoid = mybir.ActivationFunctionType.Sigmoid
    identity = mybir.ActivationFunctionType.Identity
    bf16 = mybir.dt.bfloat16

    with tile.TileContext(nc) as tc, ExitStack() as ctx:
        wpool = ctx.enter_context(tc.tile_pool(name="w", bufs=1))
        cpool = ctx.enter_context(tc.tile_pool(name="const", bufs=1))
        xpool = ctx.enter_context(tc.tile_pool(name="x", bufs=3))
        ppool = ctx.enter_context(tc.tile_pool(name="psum", bufs=8, space="PSUM"))
        spool = ctx.enter_context(tc.tile_pool(name="work", bufs=2))
        opool = ctx.enter_context(tc.tile_pool(name="out", bufs=3))
        cspool = ctx.enter_context(tc.tile_pool(name="cs", bufs=3))

        # Biases via the SWDGE queue (parallel with the big HWDGE stream):
        # [128, mh], col m = bias[m*128:(m+1)*128]
        b_sb = {}
        for x_ in names:
            t_ = cpool.tile([128, mh], f32, tag=f"b{x_}")
            nc.gpsimd.dma_start(out=t_, in_=bias[x_].rearrange("(m p) -> p m", p=128))
            b_sb[x_] = t_

        def load_x(bi, n, n_dmas=1):
            # one whole-unit tile; n_dmas>1 splits the transfer so early
            # chunks land (and unblock matmuls) sooner
            t_ = xpool.tile([128, kc, tt], mm_dt, tag="xt")
            xsrc = X5[bi][n].rearrange("p j c t -> p (j c) t")
            step = kc // n_dmas
            for c0 in range(0, kc, step):
                nc.sync.dma_start(
                    out=t_[:, c0:c0 + step, :], in_=xsrc[:, c0:c0 + step, :]
                )
            return t_

        # Consumption-ordered prologue: unit (0,0)'s x per-chunk interleaved
        # with Wq chunks (the first chain's operands), then Wk/Wv/Wg, then
        # units (0,1)/(0,2) x.
        w_sb = {}
        for x_ in names:
            t_ = wpool.tile([128, kc, e], mm_dt, tag=f"w{x_}")
            w_sb[x_] = t_

        # single sync-queue prologue in consumption order: x(0,0) quarters
        # interleaved with Wq quarters, then Wk/Wv/Wg
        x_first = xpool.tile([128, kc, tt], mm_dt, tag="xt")
        x0src = X5[0][0].rearrange("p j c t -> p (j c) t")
        for j in range(NXJ):
            nc.sync.dma_start(
                out=x_first[:, j * XJ:(j + 1) * XJ, :],
                in_=x0src[:, j * XJ:(j + 1) * XJ, :],
            )
            nc.sync.dma_start(
                out=w_sb["q"][:, j * XJ:(j + 1) * XJ, :],
                in_=W5["q"][:, j * XJ:(j + 1) * XJ, :],
            )
        for x_ in "kvg":
            for j in range(NXJ):
                nc.sync.dma_start(
                    out=w_sb[x_][:, j * XJ:(j + 1) * XJ, :],
                    in_=W5[x_][:, j * XJ:(j + 1) * XJ, :],
                )

        def emit_unit(bi, n, xt, t0, tw, cs_prev, first_sub):
            """Matmul chains + vector chains for token range [t0, t0+tw) of
            unit (bi, n). cs_prev: per-m carry tiles (or None at seq start)."""
            ps = {}
            for m in range(mh):
                for x_ in names:
                    p_ = ppool.tile([128, tw], f32, tag="ps")
                    for c in range(kc):
                        nc.tensor.matmul(
                            p_[:],
                            lhsT=w_sb[x_][:, c, m * 128:(m + 1) * 128],
                            rhs=xt[:, c, t0:t0 + tw],
                            start=(c == 0),
                            stop=(c == kc - 1),
                        )
                    ps[x_, m] = p_

            for m in range(mh):
                k_sb = spool.tile([128, tw], bf16, tag="k")
                nc.scalar.activation(
                    k_sb[:], ps["k", m][:], identity,
                    bias=b_sb["k"][:, m:m + 1], scale=1.0,
                )
                v_sb = spool.tile([128, tw], bf16, tag="v")
                nc.scalar.activation(
                    v_sb[:], ps["v", m][:], identity,
                    bias=b_sb["v"][:, m:m + 1], scale=1.0,
                )
                q_sb = spool.tile([128, tw], bf16, tag="q")
                nc.scalar.activation(
                    q_sb[:], ps["q", m][:], identity,
                    bias=b_sb["q"][:, m:m + 1], scale=1.0,
                )
                g_sb = spool.tile([128, tw], bf16, tag="g")
                nc.scalar.activation(
                    g_sb[:], ps["g", m][:], sigmoid,
                    bias=b_sb["g"][:, m:m + 1], scale=1.0,
                )
                kv = spool.tile([128, tw], bf16, tag="kv")
                nc.vector.tensor_tensor(kv[:], k_sb[:], v_sb[:], mult)
                cs = cspool.tile([128, tw], f32, tag="cs")
                init = 0.0 if first_sub else cs_prev[m][:, -1:]
                nc.vector.tensor_tensor_scan(
                    cs[:], kv[:], kv[:], init, op0=add, op1=bypass
                )
                cs_prev[m] = cs
                qg = spool.tile([128, tw], bf16, tag="qg")
                nc.gpsimd.tensor_tensor(qg[:], q_sb[:], g_sb[:], mult)
                o_sb = opool.tile([128, tw], bf16, tag="o")
                nc.vector.tensor_tensor(o_sb[:], qg[:], cs[:], mult)
                nc.sync.dma_start(
                    out=outT[bi][m * 128:(m + 1) * 128, n * tt + t0:n * tt + t0 + tw],
                    in_=o_sb[:],
                )

        for bi in range(b):
            cs_prev = [None] * mh
            for n in range(nu):
                if bi == 0 and n == 0:
                    xt = x_first
                else:
                    # 4-way split keeps chunks landing ahead of the matmul
                    # stream (a monolithic 2MB DMA starves the PE early on)
                    xt = load_x(bi, n, n_dmas=4)
                last_unit = (bi == b - 1) and (n == nu - 1)
                if last_unit:
                    # shorten the drain: final unit in 128-token sub-tiles
                    for t0 in range(0, tt, 128):
                        emit_unit(bi, n, xt, t0, 128, cs_prev,
                                  first_sub=(n == 0 and t0 == 0))
                else:
                    emit_unit(bi, n, xt, 0, tt, cs_prev, first_sub=(n == 0))

    nc.compile()
    return nc


_NC_CACHE = {}


def _get_nc():
    if "nc" not in _NC_CACHE:
        _NC_CACHE["nc"] = build_nc()
    return _NC_CACHE["nc"]


def make_in_maps(x, Wq, bq, Wk, bk, Wv, bv, Wg, bg, e=E, n_cores=N_CORES):
    # X5[b, n, p, j, c, t] = x[b, n*TT+t, (j*XJ+c)*128+p]
    X5 = np.ascontiguousarray(
        np.asarray(x, dtype=np.float32)
        .reshape(B, NU, TT, NXJ, XJ, 128)
        .transpose(0, 1, 5, 3, 4, 2)
    ).astype(MM_NP)
    Ws = {"q": Wq, "k": Wk, "v": Wv, "g": Wg}
    bs = {"q": bq, "k": bk, "v": bv, "g": bg}
    in_maps = []
    for core in range(n_cores):
        sl = slice(core * e, (core + 1) * e)
        m = {"X5": X5}
        for x_ in "qkvg":
            # W5[p, c, e] = W[sl][e, c*128+p]
            m[f"W{x_}5"] = np.ascontiguousarray(
                np.asarray(Ws[x_][sl, :], dtype=np.float32)
                .T.reshape(KC, 128, e)
                .transpose(1, 0, 2)
            ).astype(MM_NP)
            m[f"b{x_}"] = np.ascontiguousarray(np.asarray(bs[x_][sl], dtype=np.float32))
        in_maps.append(m)
    return in_maps


def gather_out(results, n_cores=N_CORES):
    # each core returns outT [B, E, S] bf16; full out = [B, S, D] fp32
    outs = [r["outT"].astype(np.float32) for r in results]
    full = np.concatenate(outs, axis=1)  # [B, D, S]
    return np.ascontiguousarray(full.transpose(0, 2, 1))


def kernel(x, Wq, bq, Wk, bk, Wv, bv, Wg, bg, **run_kwargs):
    nc = _get_nc()
    in_maps = make_in_maps(x, Wq, bq, Wk, bk, Wv, bv, Wg, bg)
    res = run_bass_kernel_spmd(
        nc, in_maps, core_ids=list(range(N_CORES)), **run_kwargs
    )
    out = gather_out(res.results)
    if run_kwargs:
        _NC_CACHE["last_result"] = res
    return out
